# revision 1
# baseline (speedup 1.0000x reference)
# Linear-chain CRF log-marginals on 8 Trainium2 NeuronCores.
#
# alpha/beta recurrences are run in the exp domain: the per-step
# LSE_k(alpha[k] + T[k,j]) becomes a matvec u @ exp(T) on the PE array
# (fp16 operands, fp32 PSUM accumulate), with a constant per-step prescale
# exp(-MU) folded into exp(scores) and a periodic data-dependent renorm to
# keep the fp16 carry in range.  The sequence is split into many short
# chunks run speculatively in lockstep (the chain mixes in ~10 steps, so a
# W-step warmup makes each chunk's carry exact up to a common constant);
# 32 chunk-scans per core share each stationary-weight load.  Chunk
# constants are resolved on the host in fp64 by matching one overlap row
# per boundary.  A second tiny device pass computes
# log(Vf*Vb) + scores + rho for the final marginals.
import numpy as np
from contextlib import ExitStack

import concourse.bass as bass
import concourse.tile as tile
from concourse import bacc, mybir
from concourse.bass_utils import run_bass_kernel_spmd

F16 = mybir.dt.float16
F32 = mybir.dt.float32
AFT = mybir.ActivationFunctionType

# ---- problem constants ----
N, C = 8192, 1024
P = 128                  # partitions
CT = C // P              # 8 contraction/output tiles of 128 classes
NCORE = 8

# ---- algorithm parameters ----
NS = 32                  # lockstep scans per core
NCH = 4 * NS             # chunks per direction (4 cores each direction)
L = N // NCH             # 64 positions per chunk
W = 32                   # warmup steps per speculative chunk
R = W + L + 1            # rows per scan: init row + L+W steps
MU = 7.927               # constant per-step log-prescale
B0 = 4.0                 # init offset: u_0 = exp(s_0 - B0)
RN = 32                  # renorm cadence (sigma measured at m, applied at m+1)
BCS = 64.0               # renorm rescale target (sum -> 64)
G = 8                    # steps per DMA/exp group
NSC = CT * NS            # 256 carry columns per core
RENORM_STEPS = list(range(RN, R - 1, RN))
NREN = max(1, len(RENORM_STEPS))

_scan_nc = None
_epi_nc = None
TIMINGS = {}


# ---------------------------------------------------------------- builders
def build_scan_nc(steps=None, repeats=1, timing_loop=None):
    nsteps = R if steps is None else steps
    nc = bacc.Bacc(None, target_bir_lowering=False)
    tmat = nc.declare_dram_parameter("tmat", [P, C * CT], F32, isOutput=False)
    srows = nc.declare_dram_parameter("srows", [P, R * NSC], F32, isOutput=False)
    vdump = nc.declare_dram_parameter("vdump", [P, R * NSC], F32, isOutput=True)
    fdump = nc.declare_dram_parameter("fdump", [1, NREN * NS], F16, isOutput=True)

    ngroups = (R + G - 1) // G

    with tile.TileContext(nc) as tc, ExitStack() as ctx:
        const = ctx.enter_context(tc.tile_pool(name="const", bufs=1))
        mpool = ctx.enter_context(tc.tile_pool(name="m16", bufs=1))
        tin = ctx.enter_context(tc.tile_pool(name="tin", bufs=2))
        spool = ctx.enter_context(tc.tile_pool(name="sin", bufs=2))
        espool = ctx.enter_context(tc.tile_pool(name="es", bufs=2))
        vpool = ctx.enter_context(tc.tile_pool(name="vst", bufs=2))
        upool = ctx.enter_context(tc.tile_pool(name="u", bufs=3))
        fpool = ctx.enter_context(tc.tile_pool(name="f", bufs=2))
        psA = ctx.enter_context(tc.tile_pool(name="psA", bufs=2, space="PSUM"))
        psB = ctx.enter_context(tc.tile_pool(name="psB", bufs=2, space="PSUM"))
        psS = ctx.enter_context(tc.tile_pool(name="psS", bufs=1, space="PSUM"))
        psC = ctx.enter_context(tc.tile_pool(name="psC", bufs=1, space="PSUM"))

        ones = const.tile([P, 1], F16)
        nc.any.memset(ones[:], 1.0)
        bc64 = const.tile([1, P], F16)
        nc.any.memset(bc64[:], BCS)
        biasmu = const.tile([P, 1], F32)
        nc.any.memset(biasmu[:], -MU)
        fexp = const.tile([1, NREN * NS], F16)
        nc.any.memset(fexp[:], 1.0)

        # M16 = exp(tmat) fp16, staged in halves through a fp32 temp
        m16 = mpool.tile([P, C * CT], F16)
        for h in range(2):
            tt = tin.tile([P, C * CT // 2], F32)
            nc.sync.dma_start(tt[:], tmat[:, h * 4096:(h + 1) * 4096])
            nc.scalar.activation(m16[:, h * 4096:(h + 1) * 4096], tt[:], AFT.Exp)

        # es groups: DMA srows slice -> exp(x - MU)
        es_tiles = [None] * ngroups

        def emit_group(g):
            lo = g * G * NSC
            hi = min(R, (g + 1) * G) * NSC
            st = spool.tile([P, G * NSC], F32)
            nc.sync.dma_start(st[:, 0:hi - lo], srows[:, lo:hi])
            et = espool.tile([P, G * NSC], F32)
            nc.scalar.activation(et[:, 0:hi - lo], st[:, 0:hi - lo], AFT.Exp,
                                 bias=biasmu[:])
            es_tiles[g] = et

        loop_cm = tc.For_i(0, timing_loop, 1) if timing_loop else ExitStack()
        with loop_cm:
            emit_group(0)

            # r = 0: V0 = es_row0 * e^(MU-B0); u0 = fp16(V0)
            vst = vpool.tile([P, G * NSC], F32)
            nc.scalar.mul(vst[:, 0:NSC], es_tiles[0][:, 0:NSC], float(np.exp(MU - B0)))
            u_prev = upool.tile([P, NSC], F16)
            nc.vector.tensor_copy(u_prev[:], vst[:, 0:NSC])

            fbc16 = None
            for r in range(1, nsteps):
                g, slot = divmod(r, G)
                if slot == 0:           # new group: fresh vst tile, prefetch es
                    if g + 1 < ngroups:
                        pass
                    if es_tiles[g] is None:
                        emit_group(g)
                    vst = vpool.tile([P, G * NSC], F32)
                if slot == 0 and g + 1 < ngroups and es_tiles[g + 1] is None:
                    emit_group(g + 1)
                es = es_tiles[g]
                off = slot * NSC

                psa = psA.tile([P, 4 * NS], F32)
                psb = psB.tile([P, 4 * NS], F32)
                for jt in range(CT):
                    tgt = psa if jt < 4 else psb
                    col = (jt % 4) * NS
                    for kt in range(CT):
                        nc.tensor.matmul(
                            tgt[:, col:col + NS],
                            m16[:, (kt * CT + jt) * P:(kt * CT + jt + 1) * P],
                            u_prev[:, kt * NS:(kt + 1) * NS],
                            start=(jt % 4 == 0 and kt == 0),
                            stop=(jt % 4 == 3 and kt == CT - 1),
                        )
                # V out (fp32) via ScalarE (fast PSUM reads)
                nc.scalar.copy(vst[:, off:off + 4 * NS], psa[:])
                nc.scalar.copy(vst[:, off + 4 * NS:off + 8 * NS], psb[:])
                # u_next = V * es  (fp16), halves so each TT waits only on ACT
                u_nxt = upool.tile([P, NSC], F16)
                nc.vector.tensor_mul(u_nxt[:, 0:4 * NS], vst[:, off:off + 4 * NS],
                                     es[:, off:off + 4 * NS])
                nc.vector.tensor_mul(u_nxt[:, 4 * NS:NSC],
                                     vst[:, off + 4 * NS:off + 8 * NS],
                                     es[:, off + 4 * NS:off + 8 * NS])
                if r - 1 in RENORM_STEPS:   # deferred renorm apply
                    nc.vector.tensor_mul(u_nxt[:], u_nxt[:], fbc16[:])
                if r in RENORM_STEPS:       # measure sigma on u_r
                    ridx = RENORM_STEPS.index(r)
                    sig = psS.tile([P, NS], F32)
                    for kt in range(CT):
                        nc.tensor.matmul(sig[0:1, :], ones[:],
                                         u_nxt[:, kt * NS:(kt + 1) * NS],
                                         start=(kt == 0), stop=(kt == CT - 1))
                    f32t = fpool.tile([1, NS], F32)
                    nc.vector.reciprocal(f32t[:], sig[0:1, :])
                    nc.vector.tensor_copy(fexp[0:1, ridx * NS:(ridx + 1) * NS],
                                          f32t[:])
                    f8 = fpool.tile([1, NSC], F16)
                    for kt in range(CT):
                        nc.vector.tensor_copy(f8[0:1, kt * NS:(kt + 1) * NS],
                                              fexp[0:1, ridx * NS:(ridx + 1) * NS])
                    pbc = psC.tile([P, NSC], F32)
                    nc.tensor.matmul(pbc[:], bc64[:], f8[:], start=True, stop=True)
                    fbc16 = fpool.tile([P, NSC], F16)
                    nc.vector.tensor_copy(fbc16[:], pbc[:])
                if slot == G - 1 or r == R - 1:
                    lo = g * G * NSC
                    hi = min(R, (g + 1) * G) * NSC
                    nc.sync.dma_start(vdump[:, lo:hi], vst[:, 0:hi - lo])
                u_prev = u_nxt


        nc.sync.dma_start(fdump[:], fexp[:])
    nc.finalize()
    return nc


def build_epi_nc():
    nc = bacc.Bacc(None, target_bir_lowering=False)
    TI = N // NCORE // P     # 8 position tiles per core
    vf = nc.declare_dram_parameter("vf", [P, TI * C], F32, isOutput=False)
    vb = nc.declare_dram_parameter("vb", [P, TI * C], F32, isOutput=False)
    sp = nc.declare_dram_parameter("sp", [P, TI * C], F32, isOutput=False)
    out = nc.declare_dram_parameter("out", [P, TI * C], F32, isOutput=True)

    with tile.TileContext(nc) as tc, ExitStack() as ctx:
        pool = ctx.enter_context(tc.tile_pool(name="sb", bufs=3))
        for ti in range(TI):
            sl = slice(ti * C, (ti + 1) * C)
            a = pool.tile([P, C], F32)
            nc.sync.dma_start(a[:], vf[:, sl])
            b = pool.tile([P, C], F32)
            nc.sync.dma_start(b[:], vb[:, sl])
            s = pool.tile([P, C], F32)
            nc.sync.dma_start(s[:], sp[:, sl])
            m = pool.tile([P, C], F32)
            nc.vector.tensor_mul(m[:], a[:], b[:])
            lg = pool.tile([P, C], F32)
            nc.scalar.activation(lg[:], m[:], AFT.Ln)
            o = pool.tile([P, C], F32)
            nc.vector.tensor_add(o[:], lg[:], s[:])
            nc.sync.dma_start(out[:, sl], o[:])
    nc.finalize()
    return nc


# ---------------------------------------------------------------- host prep
def build_chunk_scores(sdir):
    """Per-direction chunk score rows [NCH, R, C] (fp32, zero-padded)."""
    SS = np.zeros((NCH, R, C), np.float32)
    for g in range(NCH):
        lo = 0 if g == 0 else g * L - W
        seg = sdir[lo:min(lo + R, N)]
        SS[g, :seg.shape[0]] = seg
    return SS


def prep_scan_inputs(scores, T):
    maps = []
    for d, (Tp, sdir) in enumerate([(T, scores), (T.T, scores[::-1])]):
        tmat = np.ascontiguousarray(
            Tp.reshape(P, CT, P, CT).transpose(0, 1, 3, 2).reshape(P, CT * CT * P),
            dtype=np.float32)
        SS = build_chunk_scores(sdir)
        for cidx in range(4):
            SSc = SS[cidx * NS:(cidx + 1) * NS]          # [NS, R, C]
            srows = np.ascontiguousarray(
                SSc.reshape(NS, R, P, CT).transpose(2, 1, 3, 0).reshape(P, R * NSC))
            maps.append({"tmat": tmat, "srows": srows})
    return maps


def parse_scan_results(res):
    """-> per direction: Vg [NCH][R, C] fp32, logf64 [NCH][R] fp64."""
    out = []
    for d in range(2):
        Vg, logf = [], []
        for cidx in range(4):
            r = res[d * 4 + cidx]
            vd = r["vdump"].reshape(P, R, CT, NS)
            fd = r["fdump"].reshape(NREN, NS)
            for s in range(NS):
                Vg.append(np.ascontiguousarray(
                    vd[:, :, :, s].transpose(1, 0, 2).reshape(R, C)))
                lf = np.zeros(R)
                for mi, m in enumerate(RENORM_STEPS):
                    lf[m + 2:] += -np.log(BCS * np.float64(fd[mi, s]))
                logf.append(lf)
        out.append((Vg, logf))
    return out


def _cf(r, lf):
    # additive constant of alpha rows: alpha_r = log V_r + S_r*[r>0] + cf
    if r == 0:
        return B0
    return B0 + (r - 1) * MU + lf[r]


def stitch_direction(Vg, logf, sdir64):
    """-> delta [NCH] fp64, max stitch residual std (diagnostic)."""
    deltas = np.zeros(NCH)
    resid = 0.0
    prev_ovl = None
    delta = 0.0
    for g in range(NCH):
        warm = 0 if g == 0 else W
        lv = np.log(Vg[g].astype(np.float64) + 0.0)
        if g > 0:
            first = lv[warm] + sdir64[g * L] + _cf(warm, logf[g])
            dvec = prev_ovl - first
            delta = float(dvec.mean())
            resid = max(resid, float(dvec.std()))
        deltas[g] = delta
        if g + 1 < NCH:
            prev_ovl = (lv[warm + L] + sdir64[(g + 1) * L]
                        + _cf(warm + L, logf[g]) + delta)
    return deltas, resid


def host_stitch(res1, scores):
    s64 = scores.astype(np.float64)
    (Vf, lff), (Vb, lfb) = parse_scan_results(res1)
    df, rf = stitch_direction(Vf, lff, s64)
    db, rb = stitch_direction(Vb, lfb, s64[::-1])
    TIMINGS["stitch_resid"] = max(rf, rb)

    # Z from alpha row at position N-1 (last fwd chunk, r = W+L-1)
    g = NCH - 1
    aN = (np.log(Vf[g][W + L - 1].astype(np.float64)) + s64[N - 1]
          + _cf(W + L - 1, lff[g]) + df[g])
    m = aN.max()
    Z = m + np.log(np.exp(aN - m).sum())

    # per-position row constants
    i = np.arange(N)
    gf = i // L
    rfr = i - gf * L + np.where(gf > 0, W, 0)
    rev = N - 1 - i
    gb = rev // L
    rbr = rev - gb * L + np.where(gb > 0, W, 0)
    cf = np.empty(N)
    cb = np.empty(N)
    for k in range(N):
        cf[k] = _cf(rfr[k], lff[gf[k]]) + df[gf[k]]
        cb[k] = _cf(rbr[k], lfb[gb[k]]) + db[gb[k]]
    coef = ((rfr > 0).astype(np.float64) + (rbr > 0).astype(np.float64) - 1.0)
    rho = cf + cb - Z
    sp = (s64 * coef[:, None] + rho[:, None]).astype(np.float32)

    # gather V rows per position
    VF = np.empty((N, C), np.float32)
    VBr = np.empty((N, C), np.float32)
    for g in range(NCH):
        warm = 0 if g == 0 else W
        VF[g * L:(g + 1) * L] = Vf[g][warm:warm + L]
        VBr[g * L:(g + 1) * L] = Vb[g][warm:warm + L]
    VB = VBr[::-1]
    return VF, VB, sp


def prep_epi_inputs(VF, VB, sp):
    maps = []
    rows = N // NCORE
    for k in range(NCORE):
        sl = slice(k * rows, (k + 1) * rows)
        def lay(x):
            return np.ascontiguousarray(
                x[sl].reshape(rows // P, P, C).transpose(1, 0, 2)
                .reshape(P, rows * C // P))
        maps.append({"vf": lay(VF), "vb": lay(VB), "sp": lay(sp)})
    return maps


def assemble_output(res2):
    rows = N // NCORE
    out = np.empty((N, C), np.float32)
    for k in range(NCORE):
        o = res2[k]["out"].reshape(P, rows // P, C).transpose(1, 0, 2)
        out[k * rows:(k + 1) * rows] = o.reshape(rows, C)
    return out


# ---------------------------------------------------------------- emulation
def emulate_scan_core(inmap):
    tmat = inmap["tmat"]
    M16 = np.exp(tmat.astype(np.float32)).astype(np.float16)
    es = np.exp(inmap["srows"].astype(np.float32) - np.float32(MU))
    vst = np.zeros((P, R * NSC), np.float32)
    vst[:, 0:NSC] = es[:, 0:NSC] * np.float32(np.exp(MU - B0))
    u = vst[:, 0:NSC].astype(np.float16)
    fdump = np.ones((1, NREN * NS), np.float16)
    Mr = M16.astype(np.float32).reshape(P, CT, CT, P)   # [p, kt, jt, q]
    fbc = None
    for r in range(1, R):
        U = u.astype(np.float32).reshape(P, CT, NS)
        ps = np.einsum('pkjq,pks->qjs', Mr, U, optimize=True)
        ps = ps.reshape(P, NSC)
        vst[:, r * NSC:(r + 1) * NSC] = ps
        un = (ps * es[:, r * NSC:(r + 1) * NSC]).astype(np.float16)
        if r - 1 in RENORM_STEPS:
            un = (un.astype(np.float32) * fbc.astype(np.float32)).astype(np.float16)
        if r in RENORM_STEPS:
            ridx = RENORM_STEPS.index(r)
            sig = un.astype(np.float32).reshape(P, CT, NS).sum(axis=(0, 1))
            f16 = (np.float32(1.0) / sig).astype(np.float16)
            fdump[0, ridx * NS:(ridx + 1) * NS] = f16
            fb_row = (np.float32(BCS) * f16.astype(np.float32)).astype(np.float16)
            fbc = np.broadcast_to(np.tile(fb_row, CT)[None, :], (P, NSC))
        u = un
    return {"vdump": vst, "fdump": fdump}


def emulate_epi_core(inmap):
    m = inmap["vf"].astype(np.float32) * inmap["vb"].astype(np.float32)
    return {"out": np.log(m) + inmap["sp"]}


# ---------------------------------------------------------------- main entry
def kernel(scores, T, simulate=False):
    import time
    global _scan_nc, _epi_nc
    scores = np.ascontiguousarray(np.asarray(scores), dtype=np.float32)
    T = np.ascontiguousarray(np.asarray(T), dtype=np.float32)

    t0 = time.time()
    in1 = prep_scan_inputs(scores, T)
    TIMINGS["prep1"] = time.time() - t0

    t0 = time.time()
    if simulate:
        res1 = [emulate_scan_core(m) for m in in1]
    else:
        if _scan_nc is None:
            tb = time.time()
            _scan_nc = build_scan_nc()
            TIMINGS["build1"] = time.time() - tb
        res1 = run_bass_kernel_spmd(_scan_nc, in1, list(range(NCORE))).results
    TIMINGS["pass1"] = time.time() - t0

    t0 = time.time()
    VF, VB, sp = host_stitch(res1, scores)
    in2 = prep_epi_inputs(VF, VB, sp)
    TIMINGS["host"] = time.time() - t0

    t0 = time.time()
    if simulate:
        res2 = [emulate_epi_core(m) for m in in2]
    else:
        if _epi_nc is None:
            tb = time.time()
            _epi_nc = build_epi_nc()
            TIMINGS["build2"] = time.time() - tb
        res2 = run_bass_kernel_spmd(_epi_nc, in2, list(range(NCORE))).results
    TIMINGS["pass2"] = time.time() - t0

    t0 = time.time()
    out = assemble_output(res2)
    TIMINGS["asm"] = time.time() - t0
    return out



# revision 13
# speedup vs baseline: 1.9369x; 1.9369x over previous
# Linear-chain CRF log-marginals on 8 Trainium2 NeuronCores.
#
# alpha/beta recurrences run in the exp domain: the per-step
# LSE_k(alpha[k] + T[k,j]) becomes a matvec u @ exp(T) on the PE array
# (fp16 operands, fp32 PSUM accumulate), with a constant per-step prescale
# exp(-MU) folded into exp(scores) and a periodic data-dependent renorm to
# keep the fp16 carry in range.  The sequence is split into many short
# chunks run speculatively in lockstep (the chain mixes in ~10 steps, so a
# W-step warmup makes each chunk's carry exact up to a common constant);
# NS chunk-scans per core share each stationary-weight load.  Chunk
# constants are resolved on the host in fp64 by matching one overlap row
# per boundary.  The scan dumps ln(V) + s/2 (fp16) for the L+1 body rows
# only; a second tiny device pass computes dumpF + dumpB + rho for the
# final marginals.  The first/last W positions (chunk 0 of each direction
# has no warmup offset) are computed exactly on the host in fp64.
import os
import numpy as np
from contextlib import ExitStack

import concourse.bass as bass
import concourse.tile as tile
from concourse import bacc, mybir
from concourse.bass_utils import run_bass_kernel_spmd

F16 = mybir.dt.float16
F32 = mybir.dt.float32
F8 = mybir.dt.float8e4
AFT = mybir.ActivationFunctionType

# ---- problem constants ----
N, C = 8192, 1024
P = 128                  # partitions
CT = C // P              # 8 contraction/output tiles of 128 classes
NCORE = 8

# ---- algorithm parameters ----
NS = int(os.environ.get("CRF_NS", 128))  # lockstep scans per core
W = int(os.environ.get("CRF_W", 2))      # warmup steps per speculative chunk
FP8 = int(os.environ.get("CRF_FP8", 0))  # fp8e4 carry+weights (normal mode)
STRIP = int(os.environ.get("CRF_STRIP", 0))  # 1: MMs only, 2: MMs+DVE (perf debug)
NCH = 4 * NS             # chunks per direction (4 cores each direction)
L = N // NCH             # positions per chunk
R = W + L + 1            # rows per scan: init row + L+W steps
LD = L + 1               # dumped body rows per scan (rows W..W+L)
MU = 7.927               # constant per-step log-prescale
# carry centering: fp16 targets sum 64; fp8e4 needs sum ~512 so the
# lognormal tail stays above the 2^-9 subnormal floor (and max << 240)
B0 = 1.194 if FP8 else 4.0               # init offset: u_0 = exp(s_0 - B0)
RN = int(os.environ.get("CRF_RN", 8 if FP8 else 32))  # renorm cadence
BCS = 512.0 if FP8 else 64.0             # renorm target (power of 2: exact fp16)
NSC = CT * NS            # carry columns per core
G = max(2, 4096 // NSC)  # steps per DMA/exp group (SBUF-bounded)
RENORM_STEPS = list(range(RN, R - 1, RN))
NREN = max(1, len(RENORM_STEPS))

assert L >= W, (L, W)
assert N % NCH == 0

_scan_nc = None
_epi_nc = None
TIMINGS = {}


# ---------------------------------------------------------------- builders
def build_scan_nc(steps=None, repeats=1, timing_loop=None):
    if repeats != 1 and timing_loop is None:
        timing_loop = repeats
    nsteps = R if steps is None else steps
    nc = bacc.Bacc(None, target_bir_lowering=False)
    tmat = nc.declare_dram_parameter("tmat", [P, C * CT], F32, isOutput=False)
    # srows holds s/2 (host pre-halved); es = exp(2*x - MU), dump = ln(V) + x
    srows = nc.declare_dram_parameter("srows", [P, R * NSC], F32, isOutput=False)
    ldump = nc.declare_dram_parameter("ldump", [P, LD * NSC], F16, isOutput=True)
    fdump = nc.declare_dram_parameter("fdump", [1, NREN * NS], F16, isOutput=True)

    ngroups = (nsteps + G - 1) // G

    with tile.TileContext(nc) as tc, ExitStack() as ctx:
        const = ctx.enter_context(tc.tile_pool(name="const", bufs=1))
        mpool = ctx.enter_context(tc.tile_pool(name="m16", bufs=1))
        tin = ctx.enter_context(tc.tile_pool(name="tin", bufs=2))
        spool = ctx.enter_context(tc.tile_pool(name="sin", bufs=2))
        espool = ctx.enter_context(tc.tile_pool(name="es", bufs=2))
        lnpool = ctx.enter_context(tc.tile_pool(name="lnv", bufs=2))
        ltpool = ctx.enter_context(tc.tile_pool(name="lntmp", bufs=2))
        upool = ctx.enter_context(tc.tile_pool(name="u", bufs=3))
        fpool = ctx.enter_context(tc.tile_pool(name="f", bufs=2))
        psA = ctx.enter_context(tc.tile_pool(name="psA", bufs=3, space="PSUM"))
        psB = ctx.enter_context(tc.tile_pool(name="psB", bufs=3, space="PSUM"))
        psS = ctx.enter_context(tc.tile_pool(name="psS", bufs=1, space="PSUM"))
        psC = ctx.enter_context(tc.tile_pool(name="psC", bufs=1, space="PSUM"))

        UD = F8 if FP8 else F16
        ones = const.tile([P, 1], UD)
        nc.any.memset(ones[:], 1.0)
        bc64 = const.tile([1, P], F16)
        nc.any.memset(bc64[:], BCS)
        biasmu = const.tile([P, 1], F32)
        nc.any.memset(biasmu[:], -MU)
        biasb0 = const.tile([P, 1], F32)
        nc.any.memset(biasb0[:], -B0)
        fexp = const.tile([1, NREN * NS], F16)
        nc.any.memset(fexp[:], 1.0)

        # M16 = exp(tmat) fp16, staged in halves through a fp32 temp
        m16 = mpool.tile([P, C * CT], UD)
        for h in range(2):
            tt = tin.tile([P, C * CT // 2], F32)
            nc.sync.dma_start(tt[:], tmat[:, h * 4096:(h + 1) * 4096])
            nc.scalar.activation(m16[:, h * 4096:(h + 1) * 4096], tt[:], AFT.Exp)

        # per-group staging: st = s/2 (raw DMA), es = exp(2*st - MU)
        st_tiles = [None] * ngroups
        es_tiles = [None] * ngroups

        def emit_group(g):
            lo = g * G * NSC
            hi = min(nsteps, (g + 1) * G) * NSC
            st = spool.tile([P, G * NSC], F32)
            nc.sync.dma_start(st[:, 0:hi - lo], srows[:, lo:hi])
            et = espool.tile([P, G * NSC], F32)
            nc.scalar.activation(et[:, 0:hi - lo], st[:, 0:hi - lo], AFT.Exp,
                                 bias=biasmu[:], scale=2.0)
            st_tiles[g] = st
            es_tiles[g] = et

        loop_cm = tc.For_i(0, timing_loop, 1) if timing_loop else ExitStack()
        with loop_cm:
            emit_group(0)

            # r = 0: u0 = exp(2*st0 - B0) fp16  (row 0 is never dumped: W >= 1)
            u_prev = upool.tile([P, NSC], UD)
            nc.scalar.activation(u_prev[:], st_tiles[0][:, 0:NSC], AFT.Exp,
                                 bias=biasb0[:], scale=2.0)

            lnv = None
            fbc16 = None
            for r in range(1, nsteps):
                g, slot = divmod(r, G)
                if STRIP == 1:
                    g, slot = 0, 0
                if slot == 0 and es_tiles[g] is None:
                    emit_group(g)
                if slot == 0 and g + 1 < ngroups and es_tiles[g + 1] is None and not STRIP:
                    emit_group(g + 1)
                if (slot == 0 or lnv is None) and not STRIP:
                    lnv = lnpool.tile([P, G * NSC], F16)
                es = es_tiles[g]
                st = st_tiles[g]
                off = slot * NSC

                psa = psA.tile([P, 4 * NS], F32)
                psb = psB.tile([P, 4 * NS], F32)
                # bank-outer, then kt, then jt: one PSUM clear per bank
                # (start=True clears has_written for the WHOLE bank; later
                # first-writes to other quarters overwrite via unset bits),
                # and bank A's u blocks are ready while bank B still runs.
                for half in range(2):
                    tgt = psa if half == 0 else psb
                    jts = range(4 * half, 4 * half + 4)
                    for kt in range(CT):
                        for jt in jts:
                            col = (jt % 4) * NS
                            nc.tensor.matmul(
                                tgt[:, col:col + NS],
                                m16[:, (kt * CT + jt) * P:(kt * CT + jt + 1) * P],
                                u_prev[:, kt * NS:(kt + 1) * NS],
                                start=(kt == 0 and jt == 4 * half),
                                stop=(kt == CT - 1 and jt == 4 * half + 3),
                            )
                # u_next = V * es  (fp16), per dst block so TensorE restarts early
                if STRIP == 1:
                    continue
                u_nxt = upool.tile([P, NSC], UD)
                for q in range(CT):
                    src = psa if q < 4 else psb
                    col = (q % 4) * NS
                    nc.vector.tensor_mul(u_nxt[:, q * NS:(q + 1) * NS],
                                         src[:, col:col + NS],
                                         es[:, off + q * NS:off + (q + 1) * NS])
                if r - 1 in RENORM_STEPS:   # deferred renorm apply
                    nc.vector.tensor_mul(u_nxt[:], u_nxt[:], fbc16[:])
                if r in RENORM_STEPS:       # measure sigma on u_r
                    ridx = RENORM_STEPS.index(r)
                    sig = psS.tile([P, NS], F32)
                    for kt in range(CT):
                        nc.tensor.matmul(sig[0:1, :], ones[:],
                                         u_nxt[:, kt * NS:(kt + 1) * NS],
                                         start=(kt == 0), stop=(kt == CT - 1))
                    f32t = fpool.tile([1, NS], F32)
                    nc.vector.reciprocal(f32t[:], sig[0:1, :])
                    nc.vector.tensor_copy(fexp[0:1, ridx * NS:(ridx + 1) * NS],
                                          f32t[:])
                    f8 = fpool.tile([1, NSC], F16)
                    for kt in range(CT):
                        nc.vector.tensor_copy(f8[0:1, kt * NS:(kt + 1) * NS],
                                              fexp[0:1, ridx * NS:(ridx + 1) * NS])
                    pbc = psC.tile([P, NSC], F32)
                    nc.tensor.matmul(pbc[:], bc64[:], f8[:], start=True, stop=True)
                    fbc16 = fpool.tile([P, NSC], F16)
                    nc.vector.tensor_copy(fbc16[:], pbc[:])
                # body dump: ln(V) + s/2 in fp16 for rows W..W+L
                if r >= W and not STRIP:
                    lt = ltpool.tile([P, NSC], F32)
                    nc.scalar.activation(lt[:, 0:4 * NS], psa[:], AFT.Ln)
                    nc.scalar.activation(lt[:, 4 * NS:NSC], psb[:], AFT.Ln)
                    nc.vector.tensor_add(lnv[:, off:off + NSC], lt[:],
                                         st[:, off:off + NSC])
                flush = (slot == G - 1 or r == nsteps - 1) and not STRIP
                if flush and r >= W:
                    r0 = max(W, g * G)
                    nc.sync.dma_start(
                        ldump[:, (r0 - W) * NSC:(r - W + 1) * NSC],
                        lnv[:, (r0 - g * G) * NSC:(r - g * G + 1) * NSC])
                u_prev = u_nxt

        nc.sync.dma_start(fdump[:], fexp[:])
    nc.finalize()
    return nc


def build_epi_nc():
    nc = bacc.Bacc(None, target_bir_lowering=False)
    TI = N // NCORE // P     # position tiles per core
    df = nc.declare_dram_parameter("df", [P, TI * C], F16, isOutput=False)
    db = nc.declare_dram_parameter("db", [P, TI * C], F16, isOutput=False)
    rho = nc.declare_dram_parameter("rho", [P, TI], F32, isOutput=False)
    out = nc.declare_dram_parameter("out", [P, TI * C], F16, isOutput=True)

    with tile.TileContext(nc) as tc, ExitStack() as ctx:
        pool = ctx.enter_context(tc.tile_pool(name="sb", bufs=1))
        rt = pool.tile([P, TI], F32)
        nc.sync.dma_start(rt[:], rho[:])
        a = pool.tile([P, TI * C], F16)
        nc.sync.dma_start(a[:], df[:])
        b = pool.tile([P, TI * C], F16)
        nc.sync.dma_start(b[:], db[:])
        o = pool.tile([P, TI * C], F16)
        for ti in range(TI):
            sl = slice(ti * C, (ti + 1) * C)
            m = pool.tile([P, C], F32)
            nc.vector.tensor_add(m[:], a[:, sl], b[:, sl])
            nc.vector.tensor_scalar_add(o[:, sl], m[:], rt[:, ti:ti + 1])
            nc.sync.dma_start(out[:, sl], o[:, sl])
    nc.finalize()
    return nc


# ---------------------------------------------------------------- host prep
def build_chunk_scores(sdir):
    """Per-direction chunk score rows [NCH, R, C] (fp32, zero-padded)."""
    SS = np.zeros((NCH, R, C), np.float32)
    for g in range(NCH):
        lo = 0 if g == 0 else g * L - W
        seg = sdir[lo:min(lo + R, N)]
        SS[g, :seg.shape[0]] = seg
    return SS


def prep_scan_inputs(scores, T):
    maps = []
    for d, (Tp, sdir) in enumerate([(T, scores), (T.T, scores[::-1])]):
        tmat = np.ascontiguousarray(
            Tp.reshape(P, CT, P, CT).transpose(0, 1, 3, 2).reshape(P, CT * CT * P),
            dtype=np.float32)
        SS = build_chunk_scores(sdir) * np.float32(0.5)
        for cidx in range(4):
            SSc = SS[cidx * NS:(cidx + 1) * NS]          # [NS, R, C]
            srows = np.ascontiguousarray(
                SSc.reshape(NS, R, P, CT).transpose(2, 1, 3, 0).reshape(P, R * NSC))
            maps.append({"tmat": tmat, "srows": srows})
    return maps


def parse_scan_results(res):
    """-> per direction: body [NCH][LD, C] fp32 (= lnV + s/2), logf [NCH][R]."""
    out = []
    for d in range(2):
        Vg, logf = [], []
        for cidx in range(4):
            r = res[d * 4 + cidx]
            vd = r["ldump"].reshape(P, LD, CT, NS)
            fd = r["fdump"].reshape(NREN, NS)
            for s in range(NS):
                Vg.append(np.ascontiguousarray(
                    vd[:, :, :, s].transpose(1, 0, 2).reshape(LD, C))
                    .astype(np.float64))
                lf = np.zeros(R)
                for mi, m in enumerate(RENORM_STEPS):
                    lf[m + 2:] += -np.log(BCS * np.float64(fd[mi, s]))
                logf.append(lf)
        out.append((Vg, logf))
    return out


def _cf(r, lf):
    # additive constant of alpha rows: alpha_r = (body_r - s_r/2) + s_r + cf
    if r == 0:
        return B0
    return B0 + (r - 1) * MU + lf[r]


def stitch_direction(Vg, logf):
    """-> delta [NCH] fp64, max stitch residual std (diagnostic).

    Body rows are log-domain incl. s/2; the s/2 cancels in overlap diffs.
    Body index rd of chunk g corresponds to recurrence row r = W + rd.
    """
    deltas = np.zeros(NCH)
    resid = 0.0
    prev_ovl = None
    delta = 0.0
    for g in range(NCH):
        lv = Vg[g]
        if g > 0:
            first = lv[0] + _cf(W, logf[g])
            dvec = prev_ovl - first
            delta = float(dvec.mean())
            resid = max(resid, float(dvec.std()))
        deltas[g] = delta
        if g + 1 < NCH:
            rd_ovl = L if g > 0 else L - W
            prev_ovl = (lv[rd_ovl] + _cf(W + rd_ovl, logf[g]) + delta)
    return deltas, resid


def _edge_alpha(s64, T64):
    """Exact fp64 alpha rows 0..W-1 (forward convention)."""
    a = np.empty((W, C))
    a[0] = s64[0]
    for r in range(1, W):
        v = a[r - 1][:, None] + T64
        m = v.max(axis=0)
        a[r] = s64[r] + m + np.log(np.exp(v - m).sum(axis=0))
    return a


def _edge_beta(s64, T64):
    """Exact fp64 beta rows for positions N-1..N-W."""
    b = np.empty((W, C))
    b[0] = s64[N - 1]
    for r in range(1, W):
        v = T64 + b[r - 1][None, :]
        m = v.max(axis=1)
        b[r] = s64[N - 1 - r] + m + np.log(np.exp(v - m[:, None]).sum(axis=1))
    return b


def host_stitch(res1, scores, T):
    s64 = scores.astype(np.float64)
    T64 = T.astype(np.float64)
    (Vf, lff), (Vb, lfb) = parse_scan_results(res1)
    df, rf = stitch_direction(Vf, lff)
    db, rb = stitch_direction(Vb, lfb)
    TIMINGS["stitch_resid"] = max(rf, rb)

    # Z from alpha row at position N-1 (last fwd chunk, rd = L-1 -> r = W+L-1)
    g = NCH - 1
    aN = (Vf[g][L - 1] + 0.5 * s64[N - 1]
          + _cf(W + L - 1, lff[g]) + df[g])
    m = aN.max()
    Z = m + np.log(np.exp(aN - m).sum())

    # gather body rows per position + per-position row constants
    DF = np.empty((N, C), np.float32)
    DB = np.empty((N, C), np.float32)
    cf = np.empty(N)
    cb = np.empty(N)
    for g in range(NCH):
        j0 = W if g == 0 else 0          # chunk 0 covers positions W..L-1
        lo = g * L
        DF[lo + j0:lo + L] = Vf[g][0:L - j0]
        rr = W + np.arange(L - j0)
        cf[lo + j0:lo + L] = _cf_vec(rr, lff[g]) + df[g]
    for g in range(NCH):
        j0 = W if g == 0 else 0
        lo = g * L
        rows = Vb[g][0:L - j0]           # backward positions N-1-(lo+j0+k)
        idx = N - 1 - (lo + j0 + np.arange(L - j0))
        DB[idx] = rows
        cb[idx] = _cf_vec(W + np.arange(L - j0), lfb[g]) + db[g]

    # exact fp64 edges (chunk 0 of each direction has no warmup offset)
    ae = _edge_alpha(s64, T64)
    be = _edge_beta(s64, T64)
    DF[0:W] = ae - 0.5 * s64[0:W]
    cf[0:W] = 0.0
    DB[N - W:N] = be[::-1] - 0.5 * s64[N - W:N]
    cb[N - W:N] = 0.0

    rho = (cf + cb - Z).astype(np.float32)
    return DF, DB, rho


def _cf_vec(r, lf):
    return B0 + (r - 1) * MU + lf[r]


def prep_epi_inputs(DF, DB, rho):
    maps = []
    rows = N // NCORE
    TI = rows // P
    for k in range(NCORE):
        sl = slice(k * rows, (k + 1) * rows)
        def lay(x, dt):
            return np.ascontiguousarray(
                x[sl].reshape(TI, P, -1).transpose(1, 0, 2)
                .reshape(P, -1).astype(dt))
        maps.append({"df": lay(DF, np.float16), "db": lay(DB, np.float16),
                     "rho": lay(rho[:, None], np.float32)})
    return maps


def assemble_output(res2):
    rows = N // NCORE
    out = np.empty((N, C), np.float32)
    for k in range(NCORE):
        o = res2[k]["out"].reshape(P, rows // P, C).transpose(1, 0, 2)
        out[k * rows:(k + 1) * rows] = o.reshape(rows, C).astype(np.float32)
    return out


# ---------------------------------------------------------------- emulation
def emulate_scan_core(inmap):
    if FP8:
        import ml_dtypes
        UDnp = ml_dtypes.float8_e4m3   # TRN e4m3 (max 240)
    else:
        UDnp = np.float16
    tmat = inmap["tmat"]
    M16 = np.exp(tmat.astype(np.float32)).astype(UDnp)
    st = inmap["srows"].astype(np.float32)          # s/2
    es = np.exp(2.0 * st - np.float32(MU))
    u = np.exp(2.0 * st[:, 0:NSC] - np.float32(B0)).astype(UDnp)
    ldump = np.zeros((P, LD * NSC), np.float16)
    fdump = np.ones((1, NREN * NS), np.float16)
    Mr = M16.astype(np.float32).reshape(P, CT, CT, P)   # [p, kt, jt, q]
    fbc = None
    for r in range(1, R):
        U = u.astype(np.float32).reshape(P, CT, NS)
        ps = np.einsum('pkjq,pks->qjs', Mr, U, optimize=True)
        ps = ps.reshape(P, NSC)
        if r >= W:
            ldump[:, (r - W) * NSC:(r - W + 1) * NSC] = (
                np.log(ps) + st[:, r * NSC:(r + 1) * NSC])
        un = (ps * es[:, r * NSC:(r + 1) * NSC]).astype(UDnp)
        if r - 1 in RENORM_STEPS:
            un = (un.astype(np.float32) * fbc.astype(np.float32)).astype(UDnp)
        if r in RENORM_STEPS:
            ridx = RENORM_STEPS.index(r)
            sig = un.astype(np.float32).reshape(P, CT, NS).sum(axis=(0, 1))
            f16 = (np.float32(1.0) / sig).astype(np.float16)
            fdump[0, ridx * NS:(ridx + 1) * NS] = f16
            fb_row = (np.float32(BCS) * f16.astype(np.float32)).astype(np.float16)
            fbc = np.broadcast_to(np.tile(fb_row, CT)[None, :], (P, NSC))
        u = un
    return {"ldump": ldump, "fdump": fdump}


def emulate_epi_core(inmap):
    TI = N // NCORE // P
    m = inmap["df"].astype(np.float32) + inmap["db"].astype(np.float32)
    m = m.reshape(P, TI, C) + inmap["rho"].astype(np.float32)[:, :, None]
    return {"out": m.reshape(P, TI * C).astype(np.float16)}


# ---------------------------------------------------------------- main entry
def kernel(scores, T, simulate=False):
    import time
    global _scan_nc, _epi_nc
    scores = np.ascontiguousarray(np.asarray(scores), dtype=np.float32)
    T = np.ascontiguousarray(np.asarray(T), dtype=np.float32)

    t0 = time.time()
    in1 = prep_scan_inputs(scores, T)
    TIMINGS["prep1"] = time.time() - t0

    t0 = time.time()
    if simulate:
        res1 = [emulate_scan_core(m) for m in in1]
    else:
        if _scan_nc is None:
            tb = time.time()
            _scan_nc = build_scan_nc()
            TIMINGS["build1"] = time.time() - tb
        res1 = run_bass_kernel_spmd(_scan_nc, in1, list(range(NCORE))).results
    TIMINGS["pass1"] = time.time() - t0

    t0 = time.time()
    DF, DB, rho = host_stitch(res1, scores, T)
    in2 = prep_epi_inputs(DF, DB, rho)
    TIMINGS["host"] = time.time() - t0

    t0 = time.time()
    if simulate:
        res2 = [emulate_epi_core(m) for m in in2]
    else:
        if _epi_nc is None:
            tb = time.time()
            _epi_nc = build_epi_nc()
            TIMINGS["build2"] = time.time() - tb
        res2 = run_bass_kernel_spmd(_epi_nc, in2, list(range(NCORE))).results
    TIMINGS["pass2"] = time.time() - t0

    t0 = time.time()
    out = assemble_output(res2)
    TIMINGS["asm"] = time.time() - t0
    return out
